# revision 46
# baseline (speedup 1.0000x reference)
"""Bi-Real BasicBlock (binary 3x3 conv + BN(eval) + residual) on 8 TRN2 cores.

Strategy: data-parallel over batch (32 images -> 4 per core). Weights are
binarized on host (sign(W); the per-channel scale is folded into the BN
affine) and replicated to every core. x ships as fp16 (halves the input DMA;
sign() is unaffected down to |x|~3e-8 and the residual add loses <0.003
absolute vs an output scale of ~130). y ships back as fp16 (abs err ~0.06 vs
output scale ~130) halving the output DMA. On each core, per image:
  1. DMA x[b] whole-image (image 0 in three row pieces so binarization can
     start before the full image lands). Input rides the sync + gpsimd rings.
  2. ScalarE computes sign(x) -> fp8 into the interior of a zero-bordered
     [128, 58*58] padded tile.
  3. TensorE computes the 3x3 binary conv as accumulating matmuls over
     Cin=128 partitions, chunk-outer: 7 chunks of 8 output rows, each chunk
     one PSUM bank (8*58-2 = 462 matmul columns; junk at row seams skipped
     at evacuation). In fp8 mode the 9 taps run as 4 DoubleRow pair-matmuls
     (2 MACs/cycle) plus 1 normal matmul; LDWEIGHTS hides behind the
     previous 462-cycle matmul.
  4. VectorE evacuates each chunk with the BN scale and residual fused:
     out = psum * alpha + x  (scalar_tensor_tensor) -> fp16 output tile.
  5. Output DMAs one whole image at a time on the gpsimd ring; the last
     image goes out in chunk pieces across gpsimd/sync/scalar so the tail
     flush overlaps the final matmuls.
A short dummy-matmul warmup bridges from the preamble into the real stream
so the PE activity window fills and the clock gate (1.2 -> 2.4 GHz)
releases as early as possible.
"""

import os
import sys

for _p in ("/opt/trn_rl_repo", "/root/.axon_site/_ro/trn_rl_repo"):
    if os.path.isdir(_p) and _p not in sys.path:
        sys.path.append(_p)

import numpy as np
import ml_dtypes

B, CIN, H, W_, COUT = 32, 128, 56, 56, 128
HW = H * W_              # 3136
PH, PW = H + 2, W_ + 2   # 58x58 padded
N_CORES = 8
PER = B // N_CORES       # 4 images per core
CH_ROWS = 8              # output rows per PSUM chunk
N_CHUNKS = H // CH_ROWS  # 7
CHUNK = CH_ROWS * W_     # 448
NCOLS = CH_ROWS * PW - 2  # 462 matmul columns (incl. junk at row seams)
BN_EPS = 1e-5
N_WARM = int(os.environ.get("BIREAL_WARM", "33"))

MODE = os.environ.get("BIREAL_MODE", "fp8")  # "fp8" (DoubleRow) or "bf16"

# Image-0 input/sign row pieces: chunk c needs input rows [8c-1, 8c+8].
# Chunk-aligned 8-row pieces ride the sync ring in row order, each signed
# as it lands, so every chunk's sign completes ~1us before the ramping PE
# reaches it. (Concurrent queues share aggregate DMA bandwidth, so one
# row-ordered feed beats spreading image 0 across queues.)
PIECES = [(0, 9), (9, 17), (17, 25), (25, 33), (33, 41), (41, 49), (49, 56)]

# fp8 tap pairing: 9 taps in flat-offset order (kh*58+kw) are grouped into
# 4 DoubleRow pairs + 1 single. Pairs may span kernel rows: the rhs pair
# step is just the flat-offset difference.
PAIRS = [((0, 0), (0, 1)), ((0, 2), (1, 0)), ((1, 1), (1, 2)), ((2, 0), (2, 1))]
SINGLE = (2, 2)

_COMPILED = {}


def _build(has_shift):
    import concourse.bass as bass
    import concourse.tile as tile
    from concourse import bacc, mybir

    f32 = mybir.dt.float32
    f16 = mybir.dt.float16
    act_dt = mybir.dt.float8e4 if MODE == "fp8" else mybir.dt.bfloat16
    AF = mybir.ActivationFunctionType
    ALU = mybir.AluOpType

    nc = bacc.Bacc(None, target_bir_lowering=False, debug=False)

    x_d = nc.dram_tensor("x", [PER, CIN, HW], f16, kind="ExternalInput")
    if MODE == "fp8":
        wtp_d = nc.dram_tensor("wtp", [CIN, 4, 2, COUT], act_dt, kind="ExternalInput")
        wts_d = nc.dram_tensor("wts", [CIN, COUT], act_dt, kind="ExternalInput")
    else:
        wt_d = nc.dram_tensor("wt", [CIN, 9, COUT], act_dt, kind="ExternalInput")
    al_d = nc.dram_tensor("alpha", [COUT, 1], f32, kind="ExternalInput")
    al2_d = nc.dram_tensor("alpha2", [COUT, 1], f32, kind="ExternalInput")
    sh_d = nc.dram_tensor("shift", [COUT, 1], f32, kind="ExternalInput")
    y_d = nc.dram_tensor("y", [PER, COUT, HW], f16, kind="ExternalOutput")

    with tile.TileContext(nc) as tc:
        with (
            tc.tile_pool(name="consts", bufs=1) as consts,
            tc.tile_pool(name="xin", bufs=4) as xin,
            tc.tile_pool(name="acts", bufs=4) as acts,
            tc.tile_pool(name="outs", bufs=3) as outs,
            tc.tile_pool(name="psum", bufs=8, space=bass.MemorySpace.PSUM) as psum,
        ):
            # HAM warmup: near-100%-duty dummy matmuls bridging from the
            # preamble straight into the real stream, so the PE activity
            # window fills and the clock gate (1.2 -> 2.4 GHz) releases
            # early. Sized to end right as the first signed piece is ready.
            # The memset is gpsimd's FIRST op (before the weight doorbells)
            # so the first LDWEIGHTS issues as early as possible.
            warm = consts.tile([CIN, 128], act_dt)
            nc.gpsimd.memset(warm[:], 0.0)
            wps = psum.tile([64, 128], f32, tag="ps", name="warmps")
            for i in range(N_WARM):
                nc.tensor.matmul(
                    wps[:], warm[:, :64], warm[:],
                    start=(i == 0), stop=(i == N_WARM - 1),
                )

            # constants: weights ride the gpsimd ring ahead of image-1 input
            # (needed by the first real matmul); alpha/shift doorbells are
            # emitted on the scalar ring before the first sign (which is
            # data-gated anyway), landing well before the first evacuation.
            al_sb = consts.tile([COUT, 1], f32)
            al2_sb = consts.tile([COUT, 1], f32)
            sh_sb = consts.tile([COUT, 1], f32)
            if MODE == "fp8":
                wp_sb = consts.tile([CIN, 4, 2, COUT], act_dt)
                ws_sb = consts.tile([CIN, COUT], act_dt)
                nc.gpsimd.dma_start(wp_sb[:], wtp_d[:])
                nc.gpsimd.dma_start(ws_sb[:], wts_d[:])
            else:
                w_sb = consts.tile([CIN, 9, COUT], act_dt)
                nc.gpsimd.dma_start(w_sb[:], wt_d[:])

            if MODE == "fp8":
                taps = [("p", i) for i in range(len(PAIRS))] + [("s", 0)]
            else:
                taps = [("b", t) for t in range(9)]

            xts = {}
            avs = {}
            aflat = {}

            def emit_dma_memsets(b):
                # Input rides the sync ring in image/row order (one
                # full-rate row-ordered feed beats concurrent queues that
                # split the aggregate DMA bandwidth). Exception: image 1's
                # first piece rides gpsimd concurrently with image 0's
                # pieces — it must be signed before image 0's matmuls end,
                # and the sync queue can't deliver it in time.
                xt = xin.tile([CIN, HW], f16, tag="xt", name="xt")[:]
                xts[b] = xt
                if b == 0:
                    pieces = PIECES
                elif b == 1:
                    pieces = [(0, 33), (33, 56)]
                else:
                    pieces = [(0, H)]
                for pi, (r0, r1) in enumerate(pieces):
                    # image 1's second piece rides gpsimd (early, light
                    # contention); everything else queues on sync in
                    # image/row order at full rate
                    eng = nc.gpsimd if (b == 1 and pi == 1) else nc.sync
                    eng.dma_start(
                        xt[:, r0 * W_ : r1 * W_],
                        x_d[b, :, r0 * W_ : r1 * W_],
                    )
                a_sb = acts.tile([CIN, PH * PW], act_dt, name="a_sb")
                aflat[b] = a_sb[:]
                av = a_sb[:].rearrange("p (h w) -> p h w", w=PW)
                avs[b] = av
                nc.vector.memset(av[:, 0, :], 0.0)
                nc.vector.memset(av[:, PH - 1, :], 0.0)
                nc.vector.memset(av[:, 1 : PH - 1, 0:1], 0.0)
                nc.vector.memset(av[:, 1 : PH - 1, PW - 1 : PW], 0.0)
                return pieces

            def emit_sign(b, pieces):
                # Images 0,2,3: ScalarE Sign (+-1 into the padded tile).
                # Image 1: VectorE computes (x>=0)-0.5 (+-0.5 encoding; its
                # evacuation uses 2*alpha) — the vector engine is idle
                # during the ramp, so image 1's sign depends on neither the
                # scalar sign chain nor the sync input queue.
                xt, av = xts[b], avs[b]
                for r0, r1 in pieces:
                    dst = av[:, 1 + r0 : 1 + r1, 1 : 1 + W_]
                    src = xt[:, r0 * W_ : r1 * W_].rearrange(
                        "p (h w) -> p h w", w=W_
                    )
                    if b == 1:
                        nc.vector.tensor_scalar(
                            dst, src, 0.0, 0.5,
                            op0=ALU.is_ge, op1=ALU.subtract,
                        )
                    else:
                        nc.scalar.activation(dst, src, AF.Sign)

            def emit_shift(b):
                if has_shift:
                    # fold the BN shift into the residual tile in place
                    # (ordered after the sign read by the WAR dependency)
                    xt = xts[b]
                    nc.vector.tensor_scalar(xt, xt, sh_sb[:], None, op0=ALU.add)

            def emit_taps(base, targets, ncols):
                # tap-outer over `targets` = [(ps_ap, cbase), ...]: weights
                # stay loaded across the group's banks and the start-matmul
                # drain bubble is paid once per group, not per chunk
                for ti, (kind, k) in enumerate(taps):
                    start = ti == 0
                    stop = ti == len(taps) - 1
                    for ps, cbase in targets:
                        if kind == "p":
                            (ka, kb) = PAIRS[k]
                            offa = ka[0] * PW + ka[1]
                            step = kb[0] * PW + kb[1] - offa
                            rhs = bass.AP(
                                tensor=base.tensor,
                                offset=cbase + offa,
                                ap=[base.ap[0], [step, 2], [1, ncols]],
                            )
                            nc.tensor.matmul(
                                ps, wp_sb[:, k, :, :], rhs,
                                start=start, stop=stop,
                                perf_mode=mybir.MatmulPerfMode.DoubleRow,
                            )
                        else:
                            if kind == "s":
                                kh, kw = SINGLE
                                lhsT = ws_sb[:]
                            else:
                                kh, kw = divmod(k, 3)
                                lhsT = w_sb[:, k, :]
                            rhs = bass.AP(
                                tensor=base.tensor,
                                offset=cbase + kh * PW + kw,
                                ap=[base.ap[0], [1, ncols]],
                            )
                            nc.tensor.matmul(
                                ps, lhsT, rhs, start=start, stop=stop
                            )

            def emit_compute(b):
                xt = xts[b]
                base = aflat[b]
                o_sb = outs.tile([COUT, HW], f16, name="o_sb")

                # Matmul ordering. Chunk-outer (each group a single chunk)
                # exposes a ~0.4us PE-drain bubble per accumulation-group
                # start; tap-outer over a group of banks hides it (the
                # start-matmuls of a pass land on banks whose previous
                # group finished long ago) and amortizes LDWEIGHTS. Image 0
                # runs chunk-outer anyway: it is ramp/feed-limited and
                # chunk c can start as soon as sign rows reach 8c+8. The
                # last image switches back to chunk-outer for its final
                # chunks so each evacuates (and flushes) as early as
                # possible behind the last matmuls.
                if b == 0:
                    groups = [[c] for c in range(N_CHUNKS)]
                elif b == PER - 1:
                    # chunk 6 is handled separately below as two 4-row
                    # half-chunks so the final evac+DMA tail is minimal
                    groups = [[0, 1, 2, 3], [4], [5]]
                elif b == 1:
                    # matches image 1's two sign pieces
                    groups = [[0, 1, 2, 3], [4, 5, 6]]
                else:
                    # whole image signed at once -> single tap-outer group
                    # (no mid-image accumulation-group drain bubble)
                    groups = [list(range(N_CHUNKS))]

                aap = al2_sb[:] if b == 1 else al_sb[:]
                for grp in groups:
                    pss = {
                        c: psum.tile([COUT, NCOLS], f32, tag="ps", name="ps")
                        for c in grp
                    }
                    emit_taps(
                        base,
                        [
                            (pss[c][:], base.offset + CH_ROWS * c * PW)
                            for c in grp
                        ],
                        NCOLS,
                    )
                    for c in grp:
                        # evacuate on VectorE with BN scale + residual fused:
                        # out = psum * alpha + x(+shift)  (junk cols skipped)
                        psv = pss[c][:]
                        src = bass.AP(
                            tensor=psv.tensor,
                            offset=psv.offset,
                            ap=[psv.ap[0], [PW, CH_ROWS], [1, W_]],
                        )
                        csl = slice(CHUNK * c, CHUNK * (c + 1))
                        dst = o_sb[:, csl].rearrange("p (h w) -> p h w", w=W_)
                        res = xt[:, csl].rearrange("p (h w) -> p h w", w=W_)
                        nc.vector.scalar_tensor_tensor(
                            dst, src, aap, res, op0=ALU.mult, op1=ALU.add
                        )
                        # last image: flush output in chunk pieces across
                        # the sync+gpsimd rings (empty by now) so the tail
                        # overlaps the final matmuls
                        if b == PER - 1:
                            if c == 1:
                                nc.gpsimd.dma_start(
                                    y_d[b, :, : 2 * CHUNK], o_sb[:, : 2 * CHUNK]
                                )
                            elif c == 3:
                                sl = slice(2 * CHUNK, 4 * CHUNK)
                                nc.sync.dma_start(y_d[b, :, sl], o_sb[:, sl])
                            elif c == 5:
                                sl = slice(4 * CHUNK, 6 * CHUNK)
                                nc.gpsimd.dma_start(y_d[b, :, sl], o_sb[:, sl])

                if b == PER - 1:
                    # chunk 6 as two 4-row half-chunks: each is matmulled,
                    # evacuated and flushed independently, so only ~4 rows
                    # of work trail the very last matmul
                    HNC = 4 * PW - 2
                    for half in range(2):
                        ps = psum.tile([COUT, NCOLS], f32, tag="ps", name="ps")
                        hbase = base.offset + (CH_ROWS * 6 + 4 * half) * PW
                        emit_taps(base, [(ps[:, :HNC], hbase)], HNC)
                        psv = ps[:]
                        src = bass.AP(
                            tensor=psv.tensor,
                            offset=psv.offset,
                            ap=[psv.ap[0], [PW, 4], [1, W_]],
                        )
                        csl = slice(
                            CHUNK * 6 + 4 * half * W_,
                            CHUNK * 6 + (4 * half + 4) * W_,
                        )
                        dst = o_sb[:, csl].rearrange("p (h w) -> p h w", w=W_)
                        res = xt[:, csl].rearrange("p (h w) -> p h w", w=W_)
                        nc.vector.scalar_tensor_tensor(
                            dst, src, aap, res, op0=ALU.mult, op1=ALU.add
                        )
                        eng = nc.sync if half == 0 else nc.gpsimd
                        eng.dma_start(y_d[b, :, csl], o_sb[:, csl])
                else:
                    # whole-image output on the scalar ring (idle after the
                    # const doorbells), except image 1 on gpsimd which is
                    # free once image 3's input lands
                    eng = nc.gpsimd if b == 1 else nc.scalar
                    eng.dma_start(y_d[b], o_sb[:])

            # Emission (= per-engine issue) order: image 1's piece-a sign
            # lands on the vector ring BEFORE image 0's evacuations (so it
            # runs as soon as its gpsimd transfer lands, ~6us before it is
            # needed), and piece-b right after them (before image 1's own
            # evacuations need the ring).
            p0 = emit_dma_memsets(0)
            nc.scalar.dma_start(al_sb[:], al_d[:])
            nc.scalar.dma_start(al2_sb[:], al2_d[:])
            nc.scalar.dma_start(sh_sb[:], sh_d[:])
            p1 = emit_dma_memsets(1)
            emit_sign(0, p0)
            emit_shift(0)
            emit_sign(1, p1[:1])
            emit_compute(0)
            emit_sign(1, p1[1:])
            emit_shift(1)
            emit_compute(1)
            for b in (2, 3):
                pb = emit_dma_memsets(b)
                emit_sign(b, pb)
                emit_shift(b)
                emit_compute(b)

    nc.compile()
    return nc


def _get_compiled(has_shift):
    key = (MODE, bool(has_shift))
    if key not in _COMPILED:
        _COMPILED[key] = _build(has_shift)
    return _COMPILED[key]


def _prep_in_maps(x, W, gamma, beta, running_mean, running_var):
    x = np.asarray(x, dtype=np.float32)
    W = np.asarray(W, dtype=np.float32)
    gamma = np.asarray(gamma, dtype=np.float32)
    beta = np.asarray(beta, dtype=np.float32)
    running_mean = np.asarray(running_mean, dtype=np.float32)
    running_var = np.asarray(running_var, dtype=np.float32)

    scale = np.abs(W).mean(axis=(1, 2, 3))              # [Cout]
    inv = gamma / np.sqrt(running_var + BN_EPS)          # [Cout]
    alpha = (scale * inv).astype(np.float32)[:, None]    # [Cout, 1]
    shift = (beta - running_mean * inv).astype(np.float32)[:, None]

    # wsign[i, kh, kw, o] = sign(W[o, i, kh, kw])
    wsign = np.sign(W).transpose(1, 2, 3, 0)
    act_np = ml_dtypes.float8_e4m3 if MODE == "fp8" else ml_dtypes.bfloat16

    xr = np.ascontiguousarray(x.reshape(B, CIN, HW)).astype(np.float16)
    # alpha2 pairs with the +-0.5 sign encoding used for image 1
    common = {"alpha": alpha, "alpha2": (2.0 * alpha).astype(np.float32),
              "shift": shift}
    if MODE == "fp8":
        wtp = np.stack(
            [
                np.stack([wsign[:, ka[0], ka[1], :], wsign[:, kb[0], kb[1], :]], axis=1)
                for (ka, kb) in PAIRS
            ],
            axis=1,
        )  # [CIN, 4, 2, COUT]
        common["wtp"] = np.ascontiguousarray(wtp).astype(act_np)
        common["wts"] = np.ascontiguousarray(wsign[:, SINGLE[0], SINGLE[1], :]).astype(
            act_np
        )
    else:
        common["wt"] = np.ascontiguousarray(wsign.reshape(CIN, 9, COUT)).astype(act_np)

    has_shift = bool(np.any(shift != 0.0))
    in_maps = []
    for c in range(N_CORES):
        in_maps.append({"x": xr[c * PER : (c + 1) * PER], **common})
    return in_maps, has_shift


def _install_axon_trace_support():
    """Register the NTFF profiling hook that this image's antenv lacks.

    Only used by kernel_timed(); the plain kernel() path never traces.
    """
    import types

    if "antenv.axon_hooks" not in sys.modules:
        mod = types.ModuleType("antenv.axon_hooks")
        mod._hook = None

        def set_axon_ntff_profile_hook(h):
            mod._hook = h

        def get_axon_ntff_profile_hook():
            return mod._hook

        mod.set_axon_ntff_profile_hook = set_axon_ntff_profile_hook
        mod.get_axon_ntff_profile_hook = get_axon_ntff_profile_hook
        sys.modules["antenv.axon_hooks"] = mod
        import antenv

        antenv.axon_hooks = mod
    hooks = sys.modules["antenv.axon_hooks"]
    if hooks.get_axon_ntff_profile_hook() is None:
        from trn_agent_boot.trn_boot import _ntff_profile_via_ctypes

        hooks.set_axon_ntff_profile_hook(
            _ntff_profile_via_ctypes("/opt/axon/libaxon_pjrt.so")
        )
    # No S3 bucket in this sandbox; keep artifacts local.
    from concourse import bass_utils

    bass_utils.upload_artifacts = lambda tmpdir: tmpdir


def _run(in_maps, has_shift, trace=False, tmpdir=None):
    from concourse.bass_utils import run_bass_kernel_spmd

    if trace:
        _install_axon_trace_support()
    nc = _get_compiled(has_shift)
    res = run_bass_kernel_spmd(
        nc, in_maps, list(range(N_CORES)), trace=trace, tmpdir=tmpdir
    )
    y = np.concatenate([res.results[c]["y"] for c in range(N_CORES)], axis=0)
    return y.reshape(B, COUT, H, W_).astype(np.float32), res


def kernel(x, W, gamma, beta, running_mean, running_var):
    in_maps, has_shift = _prep_in_maps(x, W, gamma, beta, running_mean, running_var)
    last_err = None
    for _attempt in range(3):
        try:
            y, _ = _run(in_maps, has_shift, trace=False)
            return y
        except Exception as e:  # transient NRT device errors recover on retry
            last_err = e
    raise last_err


def kernel_timed(x, W, gamma, beta, running_mean, running_var, tmpdir=None):
    """Like kernel() but also returns the profiled HW execution time in ns."""
    in_maps, has_shift = _prep_in_maps(x, W, gamma, beta, running_mean, running_var)
    y, res = _run(in_maps, has_shift, trace=True, tmpdir=tmpdir)
    return y, res


# revision 47
# speedup vs baseline: 1.0368x; 1.0368x over previous
"""Bi-Real BasicBlock (binary 3x3 conv + BN(eval) + residual) on 8 TRN2 cores.

Strategy: data-parallel over batch (32 images -> 4 per core). Weights are
binarized on host (sign(W); the per-channel scale is folded into the BN
affine) and replicated to every core. x ships as fp16 (halves the input DMA;
sign() is unaffected down to |x|~3e-8 and the residual add loses <0.003
absolute vs an output scale of ~130). y ships back as fp16 (abs err ~0.06 vs
output scale ~130) halving the output DMA. On each core, per image:
  1. DMA x[b] whole-image (image 0 in three row pieces so binarization can
     start before the full image lands). Input rides the sync + gpsimd rings.
  2. ScalarE computes sign(x) -> fp8 into the interior of a zero-bordered
     [128, 58*58] padded tile.
  3. TensorE computes the 3x3 binary conv as accumulating matmuls over
     Cin=128 partitions, chunk-outer: 7 chunks of 8 output rows, each chunk
     one PSUM bank (8*58-2 = 462 matmul columns; junk at row seams skipped
     at evacuation). In fp8 mode the 9 taps run as 4 DoubleRow pair-matmuls
     (2 MACs/cycle) plus 1 normal matmul; LDWEIGHTS hides behind the
     previous 462-cycle matmul.
  4. VectorE evacuates each chunk with the BN scale and residual fused:
     out = psum * alpha + x  (scalar_tensor_tensor) -> fp16 output tile.
  5. Output DMAs one whole image at a time on the gpsimd ring; the last
     image goes out in chunk pieces across gpsimd/sync/scalar so the tail
     flush overlaps the final matmuls.
A short dummy-matmul warmup bridges from the preamble into the real stream
so the PE activity window fills and the clock gate (1.2 -> 2.4 GHz)
releases as early as possible.
"""

import os
import sys

for _p in ("/opt/trn_rl_repo", "/root/.axon_site/_ro/trn_rl_repo"):
    if os.path.isdir(_p) and _p not in sys.path:
        sys.path.append(_p)

import numpy as np
import ml_dtypes

B, CIN, H, W_, COUT = 32, 128, 56, 56, 128
HW = H * W_              # 3136
PH, PW = H + 2, W_ + 2   # 58x58 padded
N_CORES = 8
PER = B // N_CORES       # 4 images per core
CH_ROWS = 8              # output rows per PSUM chunk
N_CHUNKS = H // CH_ROWS  # 7
CHUNK = CH_ROWS * W_     # 448
NCOLS = CH_ROWS * PW - 2  # 462 matmul columns (incl. junk at row seams)
BN_EPS = 1e-5
N_WARM = int(os.environ.get("BIREAL_WARM", "29"))

MODE = os.environ.get("BIREAL_MODE", "fp8")  # "fp8" (DoubleRow) or "bf16"

# Image-0 input/sign row pieces: chunk c needs input rows [8c-1, 8c+8].
# Chunk-aligned 8-row pieces ride the sync ring in row order, each signed
# as it lands, so every chunk's sign completes ~1us before the ramping PE
# reaches it. (Concurrent queues share aggregate DMA bandwidth, so one
# row-ordered feed beats spreading image 0 across queues.)
PIECES = [(0, 9), (9, 17), (17, 33), (33, 49), (49, 56)]

# fp8 tap pairing: 9 taps in flat-offset order (kh*58+kw) are grouped into
# 4 DoubleRow pairs + 1 single. Pairs may span kernel rows: the rhs pair
# step is just the flat-offset difference.
PAIRS = [((0, 0), (0, 1)), ((0, 2), (1, 0)), ((1, 1), (1, 2)), ((2, 0), (2, 1))]
SINGLE = (2, 2)

_COMPILED = {}


def _build(has_shift):
    import concourse.bass as bass
    import concourse.tile as tile
    from concourse import bacc, mybir

    f32 = mybir.dt.float32
    f16 = mybir.dt.float16
    act_dt = mybir.dt.float8e4 if MODE == "fp8" else mybir.dt.bfloat16
    AF = mybir.ActivationFunctionType
    ALU = mybir.AluOpType

    nc = bacc.Bacc(None, target_bir_lowering=False, debug=False)

    x_d = nc.dram_tensor("x", [PER, CIN, HW], f16, kind="ExternalInput")
    if MODE == "fp8":
        wtp_d = nc.dram_tensor("wtp", [CIN, 4, 2, COUT], act_dt, kind="ExternalInput")
        wts_d = nc.dram_tensor("wts", [CIN, COUT], act_dt, kind="ExternalInput")
    else:
        wt_d = nc.dram_tensor("wt", [CIN, 9, COUT], act_dt, kind="ExternalInput")
    al_d = nc.dram_tensor("alpha", [COUT, 1], f32, kind="ExternalInput")
    al2_d = nc.dram_tensor("alpha2", [COUT, 1], f32, kind="ExternalInput")
    sh_d = nc.dram_tensor("shift", [COUT, 1], f32, kind="ExternalInput")
    y_d = nc.dram_tensor("y", [PER, COUT, HW], f16, kind="ExternalOutput")

    with tile.TileContext(nc) as tc:
        with (
            tc.tile_pool(name="consts", bufs=1) as consts,
            tc.tile_pool(name="xin", bufs=4) as xin,
            tc.tile_pool(name="acts", bufs=4) as acts,
            tc.tile_pool(name="outs", bufs=3) as outs,
            tc.tile_pool(name="psum", bufs=8, space=bass.MemorySpace.PSUM) as psum,
        ):
            # HAM warmup: near-100%-duty dummy matmuls bridging from the
            # preamble straight into the real stream, so the PE activity
            # window fills and the clock gate (1.2 -> 2.4 GHz) releases
            # early. Sized to end right as the first signed piece is ready.
            # The memset is gpsimd's FIRST op (before the weight doorbells)
            # so the first LDWEIGHTS issues as early as possible.
            warm = consts.tile([CIN, 128], act_dt)
            nc.gpsimd.memset(warm[:], 0.0)
            wps = psum.tile([64, 128], f32, tag="ps", name="warmps")
            for i in range(N_WARM):
                nc.tensor.matmul(
                    wps[:], warm[:, :64], warm[:],
                    start=(i == 0), stop=(i == N_WARM - 1),
                )

            # constants: weights ride the gpsimd ring ahead of image-1 input
            # (needed by the first real matmul); alpha/shift doorbells are
            # emitted on the scalar ring before the first sign (which is
            # data-gated anyway), landing well before the first evacuation.
            al_sb = consts.tile([COUT, 1], f32)
            al2_sb = consts.tile([COUT, 1], f32)
            sh_sb = consts.tile([COUT, 1], f32)
            if MODE == "fp8":
                wp_sb = consts.tile([CIN, 4, 2, COUT], act_dt)
                ws_sb = consts.tile([CIN, COUT], act_dt)
                nc.gpsimd.dma_start(wp_sb[:], wtp_d[:])
                nc.gpsimd.dma_start(ws_sb[:], wts_d[:])
            else:
                w_sb = consts.tile([CIN, 9, COUT], act_dt)
                nc.gpsimd.dma_start(w_sb[:], wt_d[:])

            if MODE == "fp8":
                taps = [("p", i) for i in range(len(PAIRS))] + [("s", 0)]
            else:
                taps = [("b", t) for t in range(9)]

            xts = {}
            avs = {}
            aflat = {}

            def emit_dma_memsets(b):
                # Input rides the sync ring in image/row order (one
                # full-rate row-ordered feed beats concurrent queues that
                # split the aggregate DMA bandwidth). Exception: image 1's
                # first piece rides gpsimd concurrently with image 0's
                # pieces — it must be signed before image 0's matmuls end,
                # and the sync queue can't deliver it in time.
                xt = xin.tile([CIN, HW], f16, tag="xt", name="xt")[:]
                xts[b] = xt
                if b == 0:
                    pieces = PIECES
                elif b == 1:
                    pieces = [(0, 33), (33, 56)]
                else:
                    pieces = [(0, H)]
                for pi, (r0, r1) in enumerate(pieces):
                    # image 1's second piece rides gpsimd (early, light
                    # contention); everything else queues on sync in
                    # image/row order at full rate
                    eng = nc.gpsimd if (b == 1 and pi == 1) else nc.sync
                    eng.dma_start(
                        xt[:, r0 * W_ : r1 * W_],
                        x_d[b, :, r0 * W_ : r1 * W_],
                    )
                a_sb = acts.tile([CIN, PH * PW], act_dt, name="a_sb")
                aflat[b] = a_sb[:]
                av = a_sb[:].rearrange("p (h w) -> p h w", w=PW)
                avs[b] = av
                nc.vector.memset(av[:, 0, :], 0.0)
                nc.vector.memset(av[:, PH - 1, :], 0.0)
                nc.vector.memset(av[:, 1 : PH - 1, 0:1], 0.0)
                nc.vector.memset(av[:, 1 : PH - 1, PW - 1 : PW], 0.0)
                return pieces

            def emit_sign(b, pieces):
                # Images 0,2,3: ScalarE Sign (+-1 into the padded tile).
                # Image 1: VectorE computes (x>=0)-0.5 (+-0.5 encoding; its
                # evacuation uses 2*alpha) — the vector engine is idle
                # during the ramp, so image 1's sign depends on neither the
                # scalar sign chain nor the sync input queue.
                xt, av = xts[b], avs[b]
                for r0, r1 in pieces:
                    dst = av[:, 1 + r0 : 1 + r1, 1 : 1 + W_]
                    src = xt[:, r0 * W_ : r1 * W_].rearrange(
                        "p (h w) -> p h w", w=W_
                    )
                    if b == 1:
                        nc.vector.tensor_scalar(
                            dst, src, 0.0, 0.5,
                            op0=ALU.is_ge, op1=ALU.subtract,
                        )
                    else:
                        nc.scalar.activation(dst, src, AF.Sign)

            def emit_shift(b):
                if has_shift:
                    # fold the BN shift into the residual tile in place
                    # (ordered after the sign read by the WAR dependency)
                    xt = xts[b]
                    nc.vector.tensor_scalar(xt, xt, sh_sb[:], None, op0=ALU.add)

            def emit_taps(base, targets, ncols):
                # tap-outer over `targets` = [(ps_ap, cbase), ...]: weights
                # stay loaded across the group's banks and the start-matmul
                # drain bubble is paid once per group, not per chunk
                for ti, (kind, k) in enumerate(taps):
                    start = ti == 0
                    stop = ti == len(taps) - 1
                    for ps, cbase in targets:
                        if kind == "p":
                            (ka, kb) = PAIRS[k]
                            offa = ka[0] * PW + ka[1]
                            step = kb[0] * PW + kb[1] - offa
                            rhs = bass.AP(
                                tensor=base.tensor,
                                offset=cbase + offa,
                                ap=[base.ap[0], [step, 2], [1, ncols]],
                            )
                            nc.tensor.matmul(
                                ps, wp_sb[:, k, :, :], rhs,
                                start=start, stop=stop,
                                perf_mode=mybir.MatmulPerfMode.DoubleRow,
                            )
                        else:
                            if kind == "s":
                                kh, kw = SINGLE
                                lhsT = ws_sb[:]
                            else:
                                kh, kw = divmod(k, 3)
                                lhsT = w_sb[:, k, :]
                            rhs = bass.AP(
                                tensor=base.tensor,
                                offset=cbase + kh * PW + kw,
                                ap=[base.ap[0], [1, ncols]],
                            )
                            nc.tensor.matmul(
                                ps, lhsT, rhs, start=start, stop=stop
                            )

            def emit_compute(b):
                xt = xts[b]
                base = aflat[b]
                o_sb = outs.tile([COUT, HW], f16, name="o_sb")

                # Matmul ordering. Chunk-outer (each group a single chunk)
                # exposes a ~0.4us PE-drain bubble per accumulation-group
                # start; tap-outer over a group of banks hides it (the
                # start-matmuls of a pass land on banks whose previous
                # group finished long ago) and amortizes LDWEIGHTS. Image 0
                # runs chunk-outer anyway: it is ramp/feed-limited and
                # chunk c can start as soon as sign rows reach 8c+8. The
                # last image switches back to chunk-outer for its final
                # chunks so each evacuates (and flushes) as early as
                # possible behind the last matmuls.
                if b == 0:
                    groups = [[c] for c in range(N_CHUNKS)]
                elif b == PER - 1:
                    # chunk 6 is handled separately below as two 4-row
                    # half-chunks so the final evac+DMA tail is minimal
                    groups = [[0, 1, 2, 3], [4], [5]]
                elif b == 1:
                    # matches image 1's two sign pieces
                    groups = [[0, 1, 2, 3], [4, 5, 6]]
                else:
                    # whole image signed at once -> single tap-outer group
                    # (no mid-image accumulation-group drain bubble)
                    groups = [list(range(N_CHUNKS))]

                aap = al2_sb[:] if b == 1 else al_sb[:]
                for grp in groups:
                    pss = {
                        c: psum.tile([COUT, NCOLS], f32, tag="ps", name="ps")
                        for c in grp
                    }
                    emit_taps(
                        base,
                        [
                            (pss[c][:], base.offset + CH_ROWS * c * PW)
                            for c in grp
                        ],
                        NCOLS,
                    )
                    for c in grp:
                        # evacuate on VectorE with BN scale + residual fused:
                        # out = psum * alpha + x(+shift)  (junk cols skipped)
                        psv = pss[c][:]
                        src = bass.AP(
                            tensor=psv.tensor,
                            offset=psv.offset,
                            ap=[psv.ap[0], [PW, CH_ROWS], [1, W_]],
                        )
                        csl = slice(CHUNK * c, CHUNK * (c + 1))
                        dst = o_sb[:, csl].rearrange("p (h w) -> p h w", w=W_)
                        res = xt[:, csl].rearrange("p (h w) -> p h w", w=W_)
                        nc.vector.scalar_tensor_tensor(
                            dst, src, aap, res, op0=ALU.mult, op1=ALU.add
                        )
                        # last image: flush output in chunk pieces across
                        # the sync+gpsimd rings (empty by now) so the tail
                        # overlaps the final matmuls
                        if b == PER - 1:
                            if c == 1:
                                nc.gpsimd.dma_start(
                                    y_d[b, :, : 2 * CHUNK], o_sb[:, : 2 * CHUNK]
                                )
                            elif c == 3:
                                sl = slice(2 * CHUNK, 4 * CHUNK)
                                nc.sync.dma_start(y_d[b, :, sl], o_sb[:, sl])
                            elif c == 5:
                                sl = slice(4 * CHUNK, 6 * CHUNK)
                                nc.gpsimd.dma_start(y_d[b, :, sl], o_sb[:, sl])

                if b == PER - 1:
                    # chunk 6 as two 4-row half-chunks: each is matmulled,
                    # evacuated and flushed independently, so only ~4 rows
                    # of work trail the very last matmul
                    HNC = 4 * PW - 2
                    for half in range(2):
                        ps = psum.tile([COUT, NCOLS], f32, tag="ps", name="ps")
                        hbase = base.offset + (CH_ROWS * 6 + 4 * half) * PW
                        emit_taps(base, [(ps[:, :HNC], hbase)], HNC)
                        psv = ps[:]
                        src = bass.AP(
                            tensor=psv.tensor,
                            offset=psv.offset,
                            ap=[psv.ap[0], [PW, 4], [1, W_]],
                        )
                        csl = slice(
                            CHUNK * 6 + 4 * half * W_,
                            CHUNK * 6 + (4 * half + 4) * W_,
                        )
                        dst = o_sb[:, csl].rearrange("p (h w) -> p h w", w=W_)
                        res = xt[:, csl].rearrange("p (h w) -> p h w", w=W_)
                        nc.vector.scalar_tensor_tensor(
                            dst, src, aap, res, op0=ALU.mult, op1=ALU.add
                        )
                        eng = nc.sync if half == 0 else nc.gpsimd
                        eng.dma_start(y_d[b, :, csl], o_sb[:, csl])
                else:
                    # whole-image output on the scalar ring (idle after the
                    # const doorbells), except image 1 on gpsimd which is
                    # free once image 3's input lands
                    eng = nc.gpsimd if b == 1 else nc.scalar
                    eng.dma_start(y_d[b], o_sb[:])

            # Emission (= per-engine issue) order: image 1's piece-a sign
            # lands on the vector ring BEFORE image 0's evacuations (so it
            # runs as soon as its gpsimd transfer lands, ~6us before it is
            # needed), and piece-b right after them (before image 1's own
            # evacuations need the ring).
            p0 = emit_dma_memsets(0)
            nc.scalar.dma_start(al_sb[:], al_d[:])
            nc.scalar.dma_start(al2_sb[:], al2_d[:])
            nc.scalar.dma_start(sh_sb[:], sh_d[:])
            p1 = emit_dma_memsets(1)
            emit_sign(0, p0)
            emit_shift(0)
            emit_sign(1, p1[:1])
            emit_compute(0)
            emit_sign(1, p1[1:])
            emit_shift(1)
            emit_compute(1)
            for b in (2, 3):
                pb = emit_dma_memsets(b)
                emit_sign(b, pb)
                emit_shift(b)
                emit_compute(b)

    nc.compile()
    return nc


def _get_compiled(has_shift):
    key = (MODE, bool(has_shift))
    if key not in _COMPILED:
        _COMPILED[key] = _build(has_shift)
    return _COMPILED[key]


def _prep_in_maps(x, W, gamma, beta, running_mean, running_var):
    x = np.asarray(x, dtype=np.float32)
    W = np.asarray(W, dtype=np.float32)
    gamma = np.asarray(gamma, dtype=np.float32)
    beta = np.asarray(beta, dtype=np.float32)
    running_mean = np.asarray(running_mean, dtype=np.float32)
    running_var = np.asarray(running_var, dtype=np.float32)

    scale = np.abs(W).mean(axis=(1, 2, 3))              # [Cout]
    inv = gamma / np.sqrt(running_var + BN_EPS)          # [Cout]
    alpha = (scale * inv).astype(np.float32)[:, None]    # [Cout, 1]
    shift = (beta - running_mean * inv).astype(np.float32)[:, None]

    # wsign[i, kh, kw, o] = sign(W[o, i, kh, kw])
    wsign = np.sign(W).transpose(1, 2, 3, 0)
    act_np = ml_dtypes.float8_e4m3 if MODE == "fp8" else ml_dtypes.bfloat16

    xr = np.ascontiguousarray(x.reshape(B, CIN, HW)).astype(np.float16)
    # alpha2 pairs with the +-0.5 sign encoding used for image 1
    common = {"alpha": alpha, "alpha2": (2.0 * alpha).astype(np.float32),
              "shift": shift}
    if MODE == "fp8":
        wtp = np.stack(
            [
                np.stack([wsign[:, ka[0], ka[1], :], wsign[:, kb[0], kb[1], :]], axis=1)
                for (ka, kb) in PAIRS
            ],
            axis=1,
        )  # [CIN, 4, 2, COUT]
        common["wtp"] = np.ascontiguousarray(wtp).astype(act_np)
        common["wts"] = np.ascontiguousarray(wsign[:, SINGLE[0], SINGLE[1], :]).astype(
            act_np
        )
    else:
        common["wt"] = np.ascontiguousarray(wsign.reshape(CIN, 9, COUT)).astype(act_np)

    has_shift = bool(np.any(shift != 0.0))
    in_maps = []
    for c in range(N_CORES):
        in_maps.append({"x": xr[c * PER : (c + 1) * PER], **common})
    return in_maps, has_shift


def _install_axon_trace_support():
    """Register the NTFF profiling hook that this image's antenv lacks.

    Only used by kernel_timed(); the plain kernel() path never traces.
    """
    import types

    if "antenv.axon_hooks" not in sys.modules:
        mod = types.ModuleType("antenv.axon_hooks")
        mod._hook = None

        def set_axon_ntff_profile_hook(h):
            mod._hook = h

        def get_axon_ntff_profile_hook():
            return mod._hook

        mod.set_axon_ntff_profile_hook = set_axon_ntff_profile_hook
        mod.get_axon_ntff_profile_hook = get_axon_ntff_profile_hook
        sys.modules["antenv.axon_hooks"] = mod
        import antenv

        antenv.axon_hooks = mod
    hooks = sys.modules["antenv.axon_hooks"]
    if hooks.get_axon_ntff_profile_hook() is None:
        from trn_agent_boot.trn_boot import _ntff_profile_via_ctypes

        hooks.set_axon_ntff_profile_hook(
            _ntff_profile_via_ctypes("/opt/axon/libaxon_pjrt.so")
        )
    # No S3 bucket in this sandbox; keep artifacts local.
    from concourse import bass_utils

    bass_utils.upload_artifacts = lambda tmpdir: tmpdir


def _run(in_maps, has_shift, trace=False, tmpdir=None):
    from concourse.bass_utils import run_bass_kernel_spmd

    if trace:
        _install_axon_trace_support()
    nc = _get_compiled(has_shift)
    res = run_bass_kernel_spmd(
        nc, in_maps, list(range(N_CORES)), trace=trace, tmpdir=tmpdir
    )
    y = np.concatenate([res.results[c]["y"] for c in range(N_CORES)], axis=0)
    return y.reshape(B, COUT, H, W_).astype(np.float32), res


def kernel(x, W, gamma, beta, running_mean, running_var):
    in_maps, has_shift = _prep_in_maps(x, W, gamma, beta, running_mean, running_var)
    last_err = None
    for _attempt in range(3):
        try:
            y, _ = _run(in_maps, has_shift, trace=False)
            return y
        except Exception as e:  # transient NRT device errors recover on retry
            last_err = e
    raise last_err


def kernel_timed(x, W, gamma, beta, running_mean, running_var, tmpdir=None):
    """Like kernel() but also returns the profiled HW execution time in ns."""
    in_maps, has_shift = _prep_in_maps(x, W, gamma, beta, running_mean, running_var)
    y, res = _run(in_maps, has_shift, trace=True, tmpdir=tmpdir)
    return y, res
